# revision 5
# baseline (speedup 1.0000x reference)
"""Causal self-attention Trainium2 Bass kernel.

Shapes (hardcoded): x [8, 2048, 126] f32, w_attn [126, 378] f32, w_proj [126, 126] f32.
Sharding: data-parallel over batch — one batch element per NeuronCore (8 cores),
no collectives; each core computes its full batch element.

Per-core algorithm (batch b, T=2048, H=6 heads, head_dim=21):
  Phase 0 (as before): xT via PE transpose; qT/kT = W^T x^T in float32r,
  replicated on partition groups 0/32/64/96; v1 = x @ w_v (bf16) with 21
  ONES columns at 32..52 per (head, k-tile) so the PV matmul emits the
  softmax denominator already replicated across 21 PSUM partitions.

  Phase 1 is j-major (q-block outer, head inner, k-group inner) and
  software-pipelined so ScalarE exp runs back-to-back while PE fills the
  next chunk's scores and the previous chunk's PV:
    chunk (h,g,j): S^T[k,q] for k-group g (4 k-tiles, tile_position row
    strips) into one 4-bank PSUM tile; exp is TWO half-chunk activations
    (banks 0-1 / 2-3) so the next chunk's score matmuls can reuse banks as
    soon as each half is drained.  PV of the previous chunk is interleaved
    between the score matmuls at half-chunk granularity.  Diagonal chunks
    (g==j) get per-strip causal mask multiplies (DVE, bf16).
    po[0:21]=PV, po[32:53]=denominator -> ONE DVE divide per (h,j) writes
    the normalized head output; out projection per q-block.
"""

import numpy as np

import concourse.bacc as bacc
import concourse.mybir as mybir
import concourse.tile as tile
from concourse import bass_utils
from concourse.masks import make_identity

B, T, C = 8, 2048, 126
H, D = 6, 21
P = 128
NT = T // P        # 16 k-tiles / q-tiles of 128
NB = T // 512      # 4 q-blocks of 512
SCALE = float(1.0 / np.sqrt(np.float32(D)))
F32 = mybir.dt.float32
F32R = mybir.dt.float32r
BF16 = mybir.dt.bfloat16
EXP = mybir.ActivationFunctionType.Exp
MULT = mybir.AluOpType.mult
DIV = mybir.AluOpType.divide
VW = 53            # v1 free width: v at 0:21, ones at 32:53


def _emit(tc, nc, x, wa_d, wp_d, out):
    with tc.tile_pool(name="persist", bufs=1) as pp:
        qT = pp.tile([P, H, T], F32R)
        kT = pp.tile([P, H, T], F32R)
        # v per head/k-tile, bf16; cols 0:21 = v, 21:32 zero, 32:53 ones
        v1 = pp.tile([P, H, NT, VW], BF16)
        outF = pp.tile([C, T], F32)
        wp_sb = pp.tile([C, C], F32)
        # mask01[k, r, c] = 1 if c >= 128*r + k else 0 (c = q offset in block)
        mask01 = pp.tile([P, 4, 512], BF16)

        # ---------------- Phase 0: load, transpose, projections ----------------
        with tc.tile_pool(name="ph0", bufs=1) as p0, \
             tc.tile_pool(name="ph0s", bufs=2) as p0s, \
             tc.tile_pool(name="ps0", bufs=2, space="PSUM") as ps0:
            x_sb = p0.tile([P, NT, C], F32)
            for t in range(NT):
                eng = nc.sync if t % 2 == 0 else nc.gpsimd
                eng.dma_start(x_sb[:, t, :], x[t * P:(t + 1) * P, :])
            wa = p0.tile([C, 3 * C], F32)
            nc.sync.dma_start(wa[:], wa_d)
            nc.sync.dma_start(wp_sb[:], wp_d)
            wa_r = p0.tile([C, 3 * C], F32R)
            nc.vector.tensor_copy(wa_r[:], wa[:])
            ident = p0.tile([P, P], F32)
            make_identity(nc, ident[:])
            nc.gpsimd.memset(mask01[:], 1.0)
            nc.gpsimd.affine_select(
                out=mask01[:], in_=mask01[:],
                compare_op=mybir.AluOpType.is_ge, fill=0.0,
                base=0, pattern=[[-P, 4], [1, 512]], channel_multiplier=-1,
            )

            nc.gpsimd.memset(v1[:, :, :, D:32], 0.0)
            nc.gpsimd.memset(v1[:, :, :, 32:VW], 1.0)
            xT_r = p0.tile([C, T], F32R)
            stgs = {s: p0s.tile([C, T], F32R, tag=f"stg{s}", bufs=1, name=f"stg{s}")
                    for s in (0, 1)}
            for nb in range(NB):
                for t in range(4 * nb, 4 * nb + 4):
                    pst = ps0.tile([C, P], F32, tag="tr")
                    nc.tensor.transpose(pst[:], x_sb[:, t, :], ident[:])
                    nc.vector.tensor_copy(xT_r[:, t * P:(t + 1) * P], pst[:])
                for s in (0, 1):
                    psqk = ps0.tile([C, 512], F32, tag="qk")
                    nc.tensor.matmul(psqk[:], wa_r[:, s * C:(s + 1) * C],
                                     xT_r[:, nb * 512:(nb + 1) * 512],
                                     start=True, stop=True)
                    nc.vector.tensor_copy(stgs[s][:, nb * 512:(nb + 1) * 512],
                                          psqk[:])
                if nb % 2 == 1:
                    half = nb // 2
                    hblk = slice(half * 1024, (half + 1) * 1024)
                    for s, dst in ((0, qT), (1, kT)):
                        for h in range(H):
                            nc.sync.dma_start(dst[0:D, h, hblk],
                                              stgs[s][h * D:(h + 1) * D, hblk])
                    for nb2 in (nb - 1, nb):
                        blk = slice(nb2 * 512, (nb2 + 1) * 512)
                        for dst in (qT, kT):
                            for r in range(1, 4):
                                nc.gpsimd.dma_start(
                                    dst[32 * r:32 * r + D, :, blk],
                                    dst[0:D, :, blk])

            for t in range(NT):
                psv = ps0.tile([P, C], F32, tag="v")
                nc.tensor.matmul(psv[:], xT_r[:, t * P:(t + 1) * P],
                                 wa_r[:, 2 * C:3 * C], start=True, stop=True)
                nc.vector.tensor_copy(
                    v1[:, :, t, 0:D],
                    psv[:].rearrange("p (h d) -> p h d", h=H),
                )

        # ---------------- Phase 1: attention, j-major pipelined ----------------
        with tc.tile_pool(name="pt", bufs=1) as ptp, \
             tc.tile_pool(name="nrm", bufs=2) as nrm, \
             tc.tile_pool(name="st", bufs=1, space="PSUM") as stp, \
             tc.tile_pool(name="po", bufs=1, space="PSUM") as pop:
            out_tiled = out.rearrange("(t p) c -> p t c", p=P)
            y_sb = nrm.tile([P, NT, C], F32, tag="y", bufs=1)

            for j in range(NB):
                q0 = 512 * j
                # chunk list for this q-block: heads outer, k-groups inner
                chunks = [(h, g) for h in range(H) for g in range(j + 1)]
                prev = None  # (h, g, pt) awaiting PV
                pos = {}     # h -> po tile

                def do_pv(ph, pg, ppt, rr, j=j):
                    po = pos[ph]
                    for r in rr:
                        kc = 4 * pg + r
                        nc.tensor.matmul(po[0:VW, :], v1[:, ph, kc, :],
                                         ppt[:, r, :],
                                         start=(kc == 0), stop=(kc == 4 * j + 3),
                                         skip_group_check=True)

                def do_div(ph, j=j, q0=q0):
                    # two reads of one PSUM bank in a single DVE op are
                    # illegal (and gpsimd cannot touch PSUM), so: reciprocal
                    # of the replicated sums -> SBUF, then multiply
                    po = pos[ph]
                    rc = nrm.tile([D, 512], F32, tag="rc", bufs=2)
                    nc.vector.reciprocal(rc[:], po[32:VW, :])
                    ot = nrm.tile([D, 512], F32, tag="ot", bufs=2)
                    nc.vector.tensor_tensor(ot[:], po[0:D, :], rc[:], op=MULT)
                    nc.sync.dma_start(outF[ph * D:(ph + 1) * D, q0:q0 + 512], ot[:])

                for (h, g) in chunks:
                    if g == 0:
                        pos[h] = pop.tile([64, 512], F32, tag="po", bufs=3,
                                          name=f"po{j}_{h}")
                    st = stp.tile([P, 4, 512], F32, tag="st", bufs=1)
                    pt = ptp.tile([P, 4, 512], BF16, tag="pt", bufs=2)
                    for half in (0, 1):
                        rr = (0, 1) if half == 0 else (2, 3)
                        if prev is not None:
                            do_pv(prev[0], prev[1], prev[2], rr)
                        for r in rr:
                            kc = 4 * g + r
                            nc.tensor.matmul(
                                st[:, r, :],
                                kT[32 * r:32 * r + D, h, kc * P:(kc + 1) * P],
                                qT[32 * r:32 * r + D, h, q0:q0 + 512],
                                start=True, stop=True,
                                tile_position=(32 * r, 0))
                        nc.scalar.activation(pt[:, rr[0]:rr[1] + 1, :],
                                             st[:, rr[0]:rr[1] + 1, :],
                                             EXP, bias=0.0, scale=SCALE)
                        if g == j:  # diagonal: causal mask per strip
                            for r in rr:
                                w = P * (r + 1)
                                nc.vector.tensor_tensor(
                                    pt[:, r, 0:w], pt[:, r, 0:w],
                                    mask01[:, r, 0:w], op=MULT)
                        if prev is not None and half == 1 and prev[1] == j:
                            do_div(prev[0])
                    prev = (h, g, pt)

                # flush last chunk of the block (always diagonal: g == j)
                do_pv(prev[0], prev[1], prev[2], (0, 1, 2, 3))
                do_div(prev[0])
                prev = None

                # out projection for this q-block's 4 q-tiles
                for t in range(4 * j, 4 * j + 4):
                    py = pop.tile([P, C], F32, tag="py", bufs=1)
                    nc.tensor.matmul(py[:], outF[:, t * P:(t + 1) * P], wp_sb[:],
                                     start=True, stop=True)
                    nc.vector.tensor_copy(y_sb[:, t, :], py[:])
                nc.sync.dma_start(out_tiled[:, 4 * j:4 * j + 4, :],
                                  y_sb[:, 4 * j:4 * j + 4, :])


def _build():
    nc = bacc.Bacc("TRN2", target_bir_lowering=False, debug=False, num_devices=B)
    x = nc.dram_tensor("x", [T, C], F32, kind="ExternalInput").ap()
    wa_d = nc.dram_tensor("w_attn", [C, 3 * C], F32, kind="ExternalInput").ap()
    wp_d = nc.dram_tensor("w_proj", [C, C], F32, kind="ExternalInput").ap()
    out = nc.dram_tensor("out", [T, C], F32, kind="ExternalOutput").ap()
    with tile.TileContext(nc) as tc:
        _emit(tc, nc, x, wa_d, wp_d, out)
    nc.compile()
    return nc


_CACHE = {}


def kernel(x, w_attn, w_proj):
    x = np.asarray(x, dtype=np.float32)
    w_attn = np.asarray(w_attn, dtype=np.float32)
    w_proj = np.asarray(w_proj, dtype=np.float32)
    assert x.shape == (B, T, C) and w_attn.shape == (C, 3 * C) and w_proj.shape == (C, C)
    if "nc" not in _CACHE:
        _CACHE["nc"] = _build()
    nc = _CACHE["nc"]
    in_maps = [
        {"x": np.ascontiguousarray(x[b]), "w_attn": w_attn, "w_proj": w_proj}
        for b in range(B)
    ]
    res = bass_utils.run_bass_kernel_spmd(nc, in_maps, core_ids=list(range(B)))
    return np.stack([res.results[b]["out"] for b in range(B)], axis=0)


# revision 6
# speedup vs baseline: 1.4057x; 1.4057x over previous
"""Causal self-attention Trainium2 Bass kernel.

Shapes (hardcoded): x [8, 2048, 126] f32, w_attn [126, 378] f32, w_proj [126, 126] f32.
Sharding: data-parallel over batch — one batch element per NeuronCore (8 cores),
no collectives; each core computes its full batch element.

Per-core algorithm (batch b, T=2048, H=6 heads, head_dim=21):
  Phase 0 (as before): xT via PE transpose; qT/kT = W^T x^T in float32r,
  replicated on partition groups 0/32/64/96; v1 = x @ w_v (bf16) with 21
  ONES columns at 32..52 per (head, k-tile) so the PV matmul emits the
  softmax denominator already replicated across 21 PSUM partitions.

  Phase 1 is j-major (q-block outer, head inner, k-group inner) and
  software-pipelined so ScalarE exp runs back-to-back while PE fills the
  next chunk's scores and the previous chunk's PV:
    chunk (h,g,j): S^T[k,q] for k-group g (4 k-tiles, tile_position row
    strips) into one 4-bank PSUM tile; exp is TWO half-chunk activations
    (banks 0-1 / 2-3) so the next chunk's score matmuls can reuse banks as
    soon as each half is drained.  PV of the previous chunk is interleaved
    between the score matmuls at half-chunk granularity.  Diagonal chunks
    (g==j) get per-strip causal mask multiplies (DVE, bf16).
    po[0:21]=PV, po[32:53]=denominator -> ONE DVE divide per (h,j) writes
    the normalized head output; out projection per q-block.
"""

import numpy as np

import concourse.bacc as bacc
import concourse.mybir as mybir
import concourse.tile as tile
from concourse import bass_utils
from concourse.masks import make_identity

B, T, C = 8, 2048, 126
H, D = 6, 21
P = 128
NT = T // P        # 16 k-tiles / q-tiles of 128
NB = T // 512      # 4 q-blocks of 512
SCALE = float(1.0 / np.sqrt(np.float32(D)))
F32 = mybir.dt.float32
F32R = mybir.dt.float32r
BF16 = mybir.dt.bfloat16
EXP = mybir.ActivationFunctionType.Exp
MULT = mybir.AluOpType.mult
DIV = mybir.AluOpType.divide
VW = 53            # v1 free width: v at 0:21, ones at 32:53


def _emit(tc, nc, x, wa_d, wp_d, out):
    with tc.tile_pool(name="persist", bufs=1) as pp:
        qT = pp.tile([P, H, T], F32R)
        kT = pp.tile([P, H, T], F32R)
        # v per head/k-tile, bf16; cols 0:21 = v, 21:32 zero, 32:53 ones
        v1 = pp.tile([P, H, NT, VW], BF16)
        outF = pp.tile([C, T], F32)
        wp_sb = pp.tile([C, C], F32)
        # mask01[k, r, c] = 1 if c >= 128*r + k else 0 (c = q offset in block)
        mask01 = pp.tile([P, 4, 512], BF16)

        # ---------------- Phase 0: load, transpose, projections ----------------
        with tc.tile_pool(name="ph0", bufs=1) as p0, \
             tc.tile_pool(name="ph0s", bufs=2) as p0s, \
             tc.tile_pool(name="ps0", bufs=2, space="PSUM") as ps0:
            x_sb = p0.tile([P, NT, C], F32)
            for t in range(NT):
                eng = nc.sync if t % 2 == 0 else nc.gpsimd
                eng.dma_start(x_sb[:, t, :], x[t * P:(t + 1) * P, :])
            wa = p0.tile([C, 3 * C], F32)
            nc.sync.dma_start(wa[:], wa_d)
            nc.sync.dma_start(wp_sb[:], wp_d)
            wa_r = p0.tile([C, 3 * C], F32R)
            nc.vector.tensor_copy(wa_r[:], wa[:])
            ident = p0.tile([P, P], F32)
            make_identity(nc, ident[:])
            nc.gpsimd.memset(mask01[:], 1.0)
            nc.gpsimd.affine_select(
                out=mask01[:], in_=mask01[:],
                compare_op=mybir.AluOpType.is_ge, fill=0.0,
                base=0, pattern=[[-P, 4], [1, 512]], channel_multiplier=-1,
            )

            nc.gpsimd.memset(v1[:, :, :, D:32], 0.0)
            nc.gpsimd.memset(v1[:, :, :, 32:VW], 1.0)
            xT_r = p0.tile([C, T], F32R)
            stgs = {s: p0s.tile([C, T], F32R, tag=f"stg{s}", bufs=1, name=f"stg{s}")
                    for s in (0, 1)}
            for nb in range(NB):
                for t in range(4 * nb, 4 * nb + 4):
                    pst = ps0.tile([C, P], F32, tag="tr")
                    nc.tensor.transpose(pst[:], x_sb[:, t, :], ident[:])
                    nc.vector.tensor_copy(xT_r[:, t * P:(t + 1) * P], pst[:])
                for s in (0, 1):
                    psqk = ps0.tile([C, 512], F32, tag="qk")
                    nc.tensor.matmul(psqk[:], wa_r[:, s * C:(s + 1) * C],
                                     xT_r[:, nb * 512:(nb + 1) * 512],
                                     start=True, stop=True)
                    nc.vector.tensor_copy(stgs[s][:, nb * 512:(nb + 1) * 512],
                                          psqk[:])
                if nb % 2 == 1:
                    half = nb // 2
                    hblk = slice(half * 1024, (half + 1) * 1024)
                    for s, dst in ((0, qT), (1, kT)):
                        for h in range(H):
                            nc.sync.dma_start(dst[0:D, h, hblk],
                                              stgs[s][h * D:(h + 1) * D, hblk])
                    for nb2 in (nb - 1, nb):
                        blk = slice(nb2 * 512, (nb2 + 1) * 512)
                        for dst in (qT, kT):
                            for r in range(1, 4):
                                nc.gpsimd.dma_start(
                                    dst[32 * r:32 * r + D, :, blk],
                                    dst[0:D, :, blk])

            for t in range(NT):
                psv = ps0.tile([P, C], F32, tag="v")
                nc.tensor.matmul(psv[:], xT_r[:, t * P:(t + 1) * P],
                                 wa_r[:, 2 * C:3 * C], start=True, stop=True)
                nc.vector.tensor_copy(
                    v1[:, :, t, 0:D],
                    psv[:].rearrange("p (h d) -> p h d", h=H),
                )

        # ---------------- Phase 1: attention, j-major pipelined ----------------
        with tc.tile_pool(name="pt", bufs=1) as ptp, \
             tc.tile_pool(name="nrm", bufs=2) as nrm, \
             tc.tile_pool(name="st", bufs=1, space="PSUM") as stp, \
             tc.tile_pool(name="po", bufs=1, space="PSUM") as pop:
            out_tiled = out.rearrange("(t p) c -> p t c", p=P)
            y_sb = nrm.tile([P, NT, C], F32, tag="y", bufs=1)

            pending_proj = [None]

            def do_proj(j):
                # out projection for q-block j's 4 q-tiles; deferred into the
                # next block's chunk stream so the PE seq never head-of-line
                # blocks on the outF DMAs
                for t in range(4 * j, 4 * j + 4):
                    py = pop.tile([P, C], F32, tag="py", bufs=1, name=f"py{t}")
                    nc.tensor.matmul(py[:], outF[:, t * P:(t + 1) * P], wp_sb[:],
                                     start=True, stop=True)
                    nc.vector.tensor_copy(y_sb[:, t, :], py[:])
                nc.sync.dma_start(out_tiled[:, 4 * j:4 * j + 4, :],
                                  y_sb[:, 4 * j:4 * j + 4, :])

            for j in range(NB):
                q0 = 512 * j
                NG = 2 * (j + 1)  # 2-k-tile chunks in this q-block
                chunks = [(h, g) for h in range(H) for g in range(NG)]
                prev = None  # (h, g, pt) awaiting PV
                pos = {}     # h -> po tile

                def do_pv(ph, pg, ppt, j=j):
                    po = pos[ph]
                    for r in (0, 1):
                        kc = 2 * pg + r
                        nc.tensor.matmul(po[0:VW, :], v1[:, ph, kc, :],
                                         ppt[:, r, :],
                                         start=(kc == 0), stop=(kc == 4 * j + 3),
                                         skip_group_check=True)

                def do_div(ph, j=j, q0=q0):
                    # two reads of one PSUM bank in a single DVE op are
                    # illegal (and gpsimd cannot touch PSUM), so: reciprocal
                    # of the replicated sums -> SBUF, then multiply
                    po = pos[ph]
                    rc = nrm.tile([D, 512], F32, tag="rc", bufs=2)
                    nc.vector.reciprocal(rc[:], po[32:VW, :])
                    ot = nrm.tile([D, 512], F32, tag="ot", bufs=2)
                    nc.vector.tensor_tensor(ot[:], po[0:D, :], rc[:], op=MULT)
                    nc.sync.dma_start(outF[ph * D:(ph + 1) * D, q0:q0 + 512], ot[:])

                for ci, (h, g) in enumerate(chunks):
                    if g == 0:
                        pos[h] = pop.tile([64, 512], F32, tag="po", bufs=3,
                                          name=f"po{j}_{h}")
                    st = stp.tile([P, 2, 512], F32, tag="st", bufs=2,
                                  name=f"st{j}_{h}_{g}")
                    pt = ptp.tile([P, 2, 512], BF16, tag="pt", bufs=3,
                                  name=f"pt{j}_{h}_{g}")
                    for r in (0, 1):
                        kc = 2 * g + r
                        m = kc % 4
                        nc.tensor.matmul(
                            st[:, r, :],
                            kT[32 * m:32 * m + D, h, kc * P:(kc + 1) * P],
                            qT[32 * m:32 * m + D, h, q0:q0 + 512],
                            start=True, stop=True,
                            tile_position=(32 * m, 0))
                    nc.scalar.activation(pt[:], st[:], EXP, bias=0.0, scale=SCALE)
                    if g >= NG - 2:  # diagonal chunk: causal mask per strip
                        for r in (0, 1):
                            kc = 2 * g + r
                            m = kc % 4
                            w = P * (m + 1)
                            nc.vector.tensor_tensor(
                                pt[:, r, 0:w], pt[:, r, 0:w],
                                mask01[:, m, 0:w], op=MULT)
                    if prev is not None:
                        do_pv(prev[0], prev[1], prev[2])
                        if prev[1] == NG - 1:
                            do_div(prev[0])
                    if ci == 2 and pending_proj[0] is not None:
                        do_proj(pending_proj[0])
                        pending_proj[0] = None
                    prev = (h, g, pt)

                # flush last chunk of the block (always diagonal)
                do_pv(prev[0], prev[1], prev[2])
                do_div(prev[0])
                if pending_proj[0] is not None:
                    do_proj(pending_proj[0])
                pending_proj[0] = j
            do_proj(NB - 1)


def _build():
    nc = bacc.Bacc("TRN2", target_bir_lowering=False, debug=False, num_devices=B)
    x = nc.dram_tensor("x", [T, C], F32, kind="ExternalInput").ap()
    wa_d = nc.dram_tensor("w_attn", [C, 3 * C], F32, kind="ExternalInput").ap()
    wp_d = nc.dram_tensor("w_proj", [C, C], F32, kind="ExternalInput").ap()
    out = nc.dram_tensor("out", [T, C], F32, kind="ExternalOutput").ap()
    with tile.TileContext(nc) as tc:
        _emit(tc, nc, x, wa_d, wp_d, out)
    nc.compile()
    return nc


_CACHE = {}


def kernel(x, w_attn, w_proj):
    x = np.asarray(x, dtype=np.float32)
    w_attn = np.asarray(w_attn, dtype=np.float32)
    w_proj = np.asarray(w_proj, dtype=np.float32)
    assert x.shape == (B, T, C) and w_attn.shape == (C, 3 * C) and w_proj.shape == (C, C)
    if "nc" not in _CACHE:
        _CACHE["nc"] = _build()
    nc = _CACHE["nc"]
    in_maps = [
        {"x": np.ascontiguousarray(x[b]), "w_attn": w_attn, "w_proj": w_proj}
        for b in range(B)
    ]
    res = bass_utils.run_bass_kernel_spmd(nc, in_maps, core_ids=list(range(B)))
    return np.stack([res.results[b]["out"] for b in range(B)], axis=0)
